# revision 21
# baseline (speedup 1.0000x reference)
"""AWD-LSTM Trainium2 kernel: 3-layer LSTM (T=70, B=80, H=1150) + vocab-33278 decoder.

Strategy: data-parallel over batch across 8 NeuronCores (10 samples/core).
All on-device tensors use a transposed [feature, time*batch] layout so that
elementwise gate math runs across 128 SBUF partitions. Weights are fp16
(fast-weight-load path on the PE), accumulation is fp32 in PSUM, cell state
is fp32. Biases are folded into the GEMMs as an extra contraction row.

Per core:
  xe.T = emb[x].T  (host gather)  -> Xi0 GEMM -> 70-step recurrence (layer 0)
  -> Xi1 GEMM -> recurrence -> Xi2 GEMM -> recurrence
  -> decoder GEMM [700,1152]x[1152,33280] streaming w_dec.T from HBM.
"""

import os
import sys
import numpy as np

sys.path.insert(0, "/opt/trn_rl_repo")

NTOK, EMB, HID = 33278, 400, 1150
SEQ, BATCH = 70, 80
NCORES = 8
BSH = BATCH // NCORES          # 10 samples per core
HP = 1152                      # padded hidden (9 * 128)
KT = HP // 128                 # 9 hidden k-tiles
G = 4 * HP                     # 4608 padded gate rows
MT = G // 128                  # 36 gate m-tiles
VP = 33280                     # padded vocab (65 * 512)
K0P = 512                      # padded layer-0 contraction (400 + bias row)

_cache = {}


def _build_nc(T):
    """Build the Bass module for T timesteps (T=SEQ for real runs)."""
    import concourse.bass as bass
    import concourse.bacc as bacc
    import concourse.mybir as mybir
    from concourse import tile
    import concourse.tile_sem_assignment as _tsa

    # Cap the DMA-completion semaphore lanes so no single instruction
    # accumulates more sync waits than the ISA allows (walrus rejects
    # "too many sync wait commands" otherwise for phase-boundary DMAs).
    _tsa.NUM_HWDGE_SEMS = 3
    _tsa.NUM_SWDGE_GLOBAL_SEMS = 1

    F16 = mybir.dt.float16
    F32 = mybir.dt.float32
    N = T * BSH
    SLOTS = (T + 1) * BSH      # h archive: slot 0 = h0, slot t+1 = h_t

    nc = bacc.Bacc("TRN2", target_bir_lowering=False, debug=False)

    # ---- DRAM I/O ----
    xeT_d = nc.dram_tensor("xeT", [K0P, N], F16, kind="ExternalInput")
    wih_d = [
        nc.dram_tensor("wih0T", [K0P, G], F16, kind="ExternalInput"),
        nc.dram_tensor("wih1T", [HP, G], F16, kind="ExternalInput"),
        nc.dram_tensor("wih2T", [HP, G], F16, kind="ExternalInput"),
    ]
    whh_d = [
        nc.dram_tensor(f"whh{l}T", [HP, G], F16, kind="ExternalInput")
        for l in range(3)
    ]
    wdT_d = nc.dram_tensor("wdT", [HP, VP], F16, kind="ExternalInput")
    ones_d = nc.dram_tensor("onesrow", [2, SLOTS], F16, kind="ExternalInput")
    h0T_d = nc.dram_tensor("h0T", [3, HP, BSH], F32, kind="ExternalInput")
    c0T_d = nc.dram_tensor("c0T", [3, HP, BSH], F32, kind="ExternalInput")

    logits_d = nc.dram_tensor("logits", [N, VP], F32, kind="ExternalOutput")
    hcT_d = nc.dram_tensor("hcT", [2, 3, HP, BSH], F32, kind="ExternalOutput")

    Sig = mybir.ActivationFunctionType.Sigmoid
    Tanh = mybir.ActivationFunctionType.Tanh
    Copy = mybir.ActivationFunctionType.Copy
    ADD = mybir.AluOpType.add

    with tile.TileContext(nc) as tc:
        with (
            tc.tile_pool(name="xe", bufs=1) as p_xe,
            tc.tile_pool(name="harch", bufs=2) as p_h,
            tc.tile_pool(name="state", bufs=1) as p_state,
            tc.tile_pool(name="work", bufs=2) as p_work,
            tc.tile_pool(name="outs", bufs=3) as p_out,
            tc.tile_pool(name="ps", bufs=2, space="PSUM") as p_ps,
        ):
            # layer-0 rhs: xe.T (with bias ones-row baked in by the host)
            xeT_s = p_xe.tile([128, 4, N], F16)
            nc.sync.dma_start(
                out=xeT_s[:],
                in_=xeT_d.ap().rearrange("(k p) n -> p k n", p=128),
            )

            H_prev = None
            h2_arch = None
            lstack = tc.tile_pool(name="wmat", bufs=1)
            p_w = lstack.__enter__()
            xstack = tc.tile_pool(name="xit", bufs=1)
            p_xi = xstack.__enter__()
            for l in range(3):
                KTl = 4 if l == 0 else KT
                # ---- Xi GEMM: Xi.T[G, N] = W_ihT.T @ rhs ----
                wih_s = p_w.tile([128, KT, G], F16, tag="wmat")
                nc.sync.dma_start(
                    out=wih_s[:, :KTl, :],
                    in_=wih_d[l].ap().rearrange("(k p) g -> p k g", p=128),
                )
                if l == 0:
                    rhs_all = xeT_s
                    rhs_cols = 0  # xeT columns start at 0
                else:
                    rhs_all = H_prev
                    rhs_cols = BSH  # skip h0 slot
                xiT = p_xi.tile([128, MT, N], F16, tag="xit")
                half = (N // 2 + 7) // 8 * 8  # 352 for N=700
                for m in range(MT):
                    for c0, cw in ((0, half), (half, N - half)):
                        ps = p_ps.tile([128, 512], F32, tag="xips")
                        for k in range(KTl):
                            nc.tensor.matmul(
                                ps[:, :cw],
                                lhsT=wih_s[:, k, 128 * m : 128 * (m + 1)],
                                rhs=rhs_all[:, k, rhs_cols + c0 : rhs_cols + c0 + cw],
                                start=(k == 0),
                                stop=(k == KTl - 1),
                            )
                        nc.scalar.copy(
                            out=xiT[:, m, c0 : c0 + cw], in_=ps[:, :cw]
                        )

                # ---- recurrence ----
                whh_s = p_w.tile([128, KT, G], F16, tag="wmat")
                nc.sync.dma_start(
                    out=whh_s[:],
                    in_=whh_d[l].ap().rearrange("(k p) g -> p k g", p=128),
                )
                H = p_h.tile([128, KT, SLOTS], F16, tag="harch")
                # h0 (fp32 in DRAM) -> fp16 slot 0; gpsimd DMA casts
                nc.gpsimd.dma_start(
                    out=H[:, :, 0:BSH],
                    in_=h0T_d.ap()[l].rearrange("(k p) b -> p k b", p=128),
                )
                # ones row for bias folding (row 1150 = k 8, partition 126)
                nc.sync.dma_start(
                    out=H[126:128, KT - 1, BSH:], in_=ones_d.ap()[:, BSH:]
                )

                c_state = p_state.tile([128, KT, BSH], F32, tag="cstate")
                nc.sync.dma_start(
                    out=c_state[:],
                    in_=c0T_d.ap()[l].rearrange("(k p) b -> p k b", p=128),
                )

                for t in range(T):
                    r0 = t * BSH          # h_{t-1} slot
                    w0 = (t + 1) * BSH    # h_t slot
                    ps = p_ps.tile([128, MT * BSH], F32, tag="rps")
                    pre = ps.rearrange("p (m b) -> p m b", b=BSH)
                    for m in range(MT):
                        for k in range(KT):
                            nc.tensor.matmul(
                                ps[:, BSH * m : BSH * (m + 1)],
                                lhsT=whh_s[:, k, 128 * m : 128 * (m + 1)],
                                rhs=H[:, k, r0 : r0 + BSH],
                                start=(k == 0),
                                stop=(k == KT - 1),
                            )
                    # pre += Xi[:, :, t]  (fp32 <- psum + fp16)
                    pre_s = p_work.tile([128, 4 * KT * BSH], F32, tag="pre")
                    pre_v = pre_s.rearrange("p (m b) -> p m b", b=BSH)
                    nc.vector.tensor_tensor(
                        pre_v[:], pre[:, :MT, :], xiT[:, :, r0 : r0 + BSH], ADD
                    )
                    gs = p_work.tile([128, 4 * KT * BSH], F32, tag="gsig")
                    nc.scalar.activation(
                        gs[:, : 3 * KT * BSH], pre_s[:, : 3 * KT * BSH], Sig
                    )
                    nc.scalar.activation(
                        gs[:, 3 * KT * BSH :], pre_s[:, 3 * KT * BSH :], Tanh
                    )
                    KB = KT * BSH
                    tmp = p_work.tile([128, KB], F32, tag="tmp")
                    # c = sig(f)*c + sig(i)*tanh(g)
                    nc.vector.tensor_mul(tmp[:], gs[:, 0:KB], gs[:, 3 * KB :])
                    nc.vector.tensor_mul(
                        c_state[:], c_state[:], gs[:, KB : 2 * KB]
                    )
                    nc.vector.tensor_add(c_state[:], c_state[:], tmp[:])
                    tch = p_work.tile([128, KB], F32, tag="tch")
                    nc.scalar.activation(tch[:], c_state[:], Tanh)
                    # h = sig(o) * tanh(c); keep ones-row intact on k=8
                    ogate = gs.rearrange("p (m b) -> p m b", b=BSH)
                    tchv = tch.rearrange("p (k b) -> p k b", b=BSH)
                    nc.vector.tensor_mul(
                        H[:, 0 : KT - 1, w0 : w0 + BSH],
                        ogate[:, 2 * KT : 3 * KT - 1, :],
                        tchv[:, 0 : KT - 1, :],
                    )
                    nc.vector.tensor_mul(
                        H[0:126, KT - 1, w0 : w0 + BSH],
                        ogate[0:126, 3 * KT - 1, :],
                        tchv[0:126, KT - 1, :],
                    )
                    if t == T - 1:
                        # fp32 copy of final h for the hs output
                        hfin = p_out.tile([128, KT, BSH], F32, tag="hfin")
                        nc.vector.tensor_mul(
                            hfin[:, 0 : KT - 1, :],
                            ogate[:, 2 * KT : 3 * KT - 1, :],
                            tchv[:, 0 : KT - 1, :],
                        )
                        nc.vector.tensor_mul(
                            hfin[0:128, KT - 1, :],
                            ogate[0:128, 3 * KT - 1, :],
                            tchv[0:128, KT - 1, :],
                        )
                        nc.sync.dma_start(
                            out=hcT_d.ap()[0, l].rearrange(
                                "(k p) b -> p k b", p=128
                            ),
                            in_=hfin[:],
                        )
                        nc.sync.dma_start(
                            out=hcT_d.ap()[1, l].rearrange(
                                "(k p) b -> p k b", p=128
                            ),
                            in_=c_state[:],
                        )
                H_prev = H
                if l == 2:
                    h2_arch = H

            xstack.__exit__(None, None, None)
            lstack.__exit__(None, None, None)

            # ---- decoder: logits[N, VP] = H2.T.T @ wdT ----
            wdstack = tc.tile_pool(name="wdec", bufs=2)
            p_wd = wdstack.__enter__()
            dpstack = tc.tile_pool(name="dps", bufs=1, space="PSUM")
            p_dps = dpstack.__enter__()
            wdv = wdT_d.ap().rearrange("(k p) v -> p k v", p=128)
            CHUNK = 2048
            n_m = (N + 127) // 128
            for c2 in range(0, VP, CHUNK):
                cw = min(CHUNK, VP - c2)
                wd_t = p_wd.tile([128, KT, CHUNK], F16, tag="wd")
                nc.sync.dma_start(
                    out=wd_t[:, :, :cw], in_=wdv[:, :, c2 : c2 + cw]
                )
                for m in range(n_m):
                    rows = min(128, N - 128 * m)
                    nsub = cw // 512
                    pss = []
                    for j in range(nsub):
                        dpst = p_dps.tile([128, 512], F32, tag=f"dps{j}", name=f"dps{j}")
                        pss.append(dpst)
                    for k in range(KT):
                        for j in range(nsub):
                            nc.tensor.matmul(
                                pss[j][:rows, :],
                                lhsT=h2_arch[:, k, BSH + 128 * m : BSH + 128 * m + rows],
                                rhs=wd_t[:, k, 512 * j : 512 * (j + 1)],
                                start=(k == 0),
                                stop=(k == KT - 1),
                            )
                    for j in range(nsub):
                        st = p_out.tile([128, 512], F32, tag="lstage")
                        nc.scalar.copy(out=st[:rows, :], in_=pss[j][:rows, :])
                        nc.sync.dma_start(
                            out=logits_d.ap()[
                                128 * m : 128 * m + rows,
                                c2 + 512 * j : c2 + 512 * (j + 1),
                            ],
                            in_=st[:rows, :],
                        )
            dpstack.__exit__(None, None, None)
            wdstack.__exit__(None, None, None)
    nc.compile()
    return nc


def _pad_gates(w):
    """[4600, K] -> [4608, K] with each 1150-row gate block padded to 1152."""
    out = np.zeros((G, w.shape[1]), w.dtype)
    for g in range(4):
        out[g * HP : g * HP + HID] = w[g * HID : (g + 1) * HID]
    return out


def _pad_gates_vec(b):
    out = np.zeros((G,), b.dtype)
    for g in range(4):
        out[g * HP : g * HP + HID] = b[g * HID : (g + 1) * HID]
    return out


def _prep_inputs(T, x, h0, c0, emb, ws):
    """Host-side prep: gather, transpose, pad, cast. Returns in_maps list."""
    f16 = np.float16
    x = np.asarray(x)
    emb = np.asarray(emb, np.float32)
    xe = emb[x]                      # [T, B, E]

    shared = {}
    for l in range(3):
        w_ih, b_ih, w_hh, b_hh = ws[l]
        kin = EMB if l == 0 else HID
        kp = K0P if l == 0 else HP
        wih_p = _pad_gates(np.asarray(w_ih, np.float32))     # [G, kin]
        bias = _pad_gates_vec(np.asarray(b_ih, np.float32) + np.asarray(b_hh, np.float32))
        lhs_ih = np.zeros((kp, G), np.float32)
        lhs_ih[:kin] = wih_p.T
        lhs_ih[kin] = bias                                   # bias row
        shared[f"wih{l}T"] = lhs_ih.astype(f16)
        whh_p = _pad_gates(np.asarray(w_hh, np.float32))     # [G, HID]
        lhs_hh = np.zeros((HP, G), np.float32)
        lhs_hh[:HID] = whh_p.T
        shared[f"whh{l}T"] = lhs_hh.astype(f16)

    w_dec, b_dec = ws[3]
    wdT = np.zeros((HP, VP), np.float32)
    wdT[:HID, :NTOK] = np.asarray(w_dec, np.float32).T
    wdT[HID, :NTOK] = np.asarray(b_dec, np.float32)          # bias row
    shared["wdT"] = wdT.astype(f16)

    in_maps = []
    for c in range(NCORES):
        bs = slice(c * BSH, (c + 1) * BSH)
        m = dict(shared)
        xeT = np.zeros((K0P, T * BSH), np.float32)
        xeT[:EMB] = xe[:, bs, :].reshape(T * BSH, EMB).T
        xeT[EMB] = 1.0                                       # bias ones-row
        m["xeT"] = xeT.astype(f16)
        hT = np.zeros((3, HP, BSH), np.float32)
        cT = np.zeros((3, HP, BSH), np.float32)
        hT[:, :HID] = np.asarray(h0, np.float32)[:, bs, :].transpose(0, 2, 1)
        cT[:, :HID] = np.asarray(c0, np.float32)[:, bs, :].transpose(0, 2, 1)
        m["h0T"] = hT
        m["c0T"] = cT
        onesrow = np.zeros((2, (T + 1) * BSH), np.float16)
        onesrow[0] = 1.0
        m["onesrow"] = onesrow
        in_maps.append(m)
    return in_maps


def run(T, x, h0, c0, emb, ws, trace=False):
    from concourse import bass_utils

    key = T
    if key not in _cache:
        _cache[key] = _build_nc(T)
    nc = _cache[key]
    in_maps = _prep_inputs(T, x, h0, c0, emb, ws)
    res = bass_utils.run_bass_kernel_spmd(
        nc, in_maps, core_ids=list(range(NCORES)), trace=trace
    )
    N = T * BSH
    decoded = np.empty((T, BATCH, NTOK), np.float32)
    hs = np.empty((3, BATCH, HID), np.float32)
    cs = np.empty((3, BATCH, HID), np.float32)
    for c in range(NCORES):
        out = res.results[c]
        bs = slice(c * BSH, (c + 1) * BSH)
        decoded[:, bs, :] = out["logits"][:, :NTOK].reshape(T, BSH, NTOK)
        hct = out["hcT"]                                     # [2, 3, HP, BSH]
        hs[:, bs, :] = hct[0, :, :HID, :].transpose(0, 2, 1)
        cs[:, bs, :] = hct[1, :, :HID, :].transpose(0, 2, 1)
    return decoded, hs, cs, res


def kernel(x, h0, c0, emb,
           w_ih0, b_ih0, w_hh0, b_hh0,
           w_ih1, b_ih1, w_hh1, b_hh1,
           w_ih2, b_ih2, w_hh2, b_hh2,
           w_dec, b_dec):
    ws = [
        (w_ih0, b_ih0, w_hh0, b_hh0),
        (w_ih1, b_ih1, w_hh1, b_hh1),
        (w_ih2, b_ih2, w_hh2, b_hh2),
        (w_dec, b_dec),
    ]
    decoded, hs, cs, _ = run(SEQ, x, h0, c0, emb, ws,
                             trace=bool(int(os.environ.get("KTRACE", "0"))))
    return decoded, hs, cs
